# revision 2
# baseline (speedup 1.0000x reference)
"""BEVPool (segment-sum) Trainium2 kernel, v5.

PE-minimal variant: on this platform PE matmul execution serializes
across the 8 cores (hardware-probed), so v5 eliminates nearly all PE
work:
  - quad-row equality matrices (E00, E11, L) are built purely on the
    DVE: one PE transpose per 16-tile superchunk produces the quad rows,
    which are replicated to all 128 partitions (QROWS) with a doubling
    chain of SBUF DMAs; each E matrix is then a single tensor_scalar
    is_equal of a QROWS slice against the tile's quad column.
  - PE does only the merge matmuls (E @ x, N=64) — three per pair.
  - merged rows are quarter-placed post-merge (DVE), duplicates are
    redirected to the dump row, and each pair is one dma_scatter_add
    (CCE add-RMW, 512B quad rows) into 4 rotating DRAM tables whose WAW
    ordering makes cross-call RMWs race-free.
"""

import os
import numpy as np

import concourse.bacc as bacc
import concourse.mybir as mybir
from concourse import tile, bass
from concourse.bass_utils import run_bass_kernel_spmd

f32 = mybir.dt.float32
bf16 = mybir.dt.bfloat16
i16 = mybir.dt.int16
i32 = mybir.dt.int32
Op = mybir.AluOpType
AX = mybir.AxisListType

NP_TOTAL = 1 * 6 * 118 * 32 * 88          # 1993728 points
NCORES = 8
NP_CORE = NP_TOTAL // NCORES              # 249216 = 128 * 1947
C = 64
H = W = 360
NCELL = H * W                             # 129600
NQUAD = NCELL // 4                        # 32400
DUMP = NQUAD                              # dump row
NROW = NQUAD + 1                          # 32401 table rows
E = 4 * C                                 # 256 els/row (512B bf16)
NTAB = 4                                  # rotating tables
SC = 16                                   # tiles per superchunk

RECIP = float(np.float32(np.float32(1.0) / np.float32(0.3)))

_cache = {}


def build_program(np_core=NP_CORE, ncores=NCORES):
    ntiles = np_core // 128
    nc = bacc.Bacc("TRN2", target_bir_lowering=False, debug=False,
                   num_devices=ncores, num_swdge_queues=4)
    geom_d = nc.dram_tensor("geom", [np_core, 3], f32, kind="ExternalInput")
    x_d = nc.dram_tensor("x", [np_core, C], f32, kind="ExternalInput")
    tabs = [nc.dram_tensor(f"tab{r}", [NROW * E], bf16, kind="Internal")
            for r in range(NTAB)]
    out_d = nc.dram_tensor("tab", [NROW * E], bf16, kind="ExternalOutput")

    geom_ap = geom_d.ap()
    x_ap = x_d.ap()

    with tile.TileContext(nc) as tc:
        with (
            tc.tile_pool(name="const", bufs=1) as cpool,
            tc.tile_pool(name="work", bufs=2) as pool,
            tc.tile_pool(name="qrows", bufs=2) as qpool,
            tc.tile_pool(name="ework", bufs=3) as epool,
            tc.tile_pool(name="stage", bufs=2) as spool,
            tc.tile_pool(name="fsum", bufs=2) as fpool,
            tc.tile_pool(name="pst", bufs=2, space="PSUM") as ppoolT,
            tc.tile_pool(name="psm", bufs=2, space="PSUM") as ppoolM,
        ):
            # ---- zero the tables ----
            z = cpool.tile([128, 8192], bf16, tag="z")
            nc.vector.memset(z[:], 0.0)
            zw = NROW * E // 128  # 64802
            for t in tabs:
                tv = t.ap().rearrange("(p w) -> p w", p=128)
                off = 0
                while off < zw:
                    w = min(8192, zw - off)
                    nc.sync.dma_start(tv[:, off:off + w], z[:, :w])
                    off += w

            # ---- constants ----
            iota_i = pool.tile([128, 256], i32, tag="scrA")
            nc.gpsimd.iota(iota_i[:, :128], [[1, 128]], channel_multiplier=0)
            iota_f = pool.tile([128, 256], f32, tag="scrB")
            nc.vector.tensor_copy(iota_f[:, :128], iota_i[:, :128])
            pidx_i = pool.tile([128, 256], i32, tag="scrA")
            nc.gpsimd.iota(pidx_i[:, :1], [[0, 1]], channel_multiplier=1)
            pidx = pool.tile([128, 256], f32, tag="scrA")
            nc.vector.tensor_copy(pidx[:, :1], pidx_i[:, :1])
            ident = cpool.tile([128, 128], f32, tag="ident")
            nc.vector.tensor_scalar(ident[:], iota_f[:, :128], pidx[:, :1],
                                    None, Op.is_equal)
            ltri = cpool.tile([128, 128], bf16, tag="ltri")
            nc.vector.tensor_scalar(ltri[:], iota_f[:, :128], pidx[:, :1],
                                    None, Op.is_lt)
            # quarter-iota row: [0]*64 [1]*64 [2]*64 [3]*64
            io256 = pool.tile([128, 256], i32, tag="scrB")
            nc.gpsimd.iota(io256[:], [[1, 256]], channel_multiplier=0)
            qiof = pool.tile([128, 256], f32, tag="scrA")
            nc.vector.tensor_scalar(qiof[:], io256[:], 1.0 / 64.0, None,
                                    Op.mult)
            qio_i = pool.tile([128, 256], i32, tag="scrB")
            nc.vector.tensor_copy(qio_i[:], qiof[:])
            qiotaf = pool.tile([128, 256], f32, tag="scrA")
            nc.vector.tensor_copy(qiotaf[:], qio_i[:])
            qcorr = pool.tile([128, 256], f32, tag="scrB")
            nc.vector.tensor_tensor(qcorr[:], qiotaf[:], qiof[:], Op.is_gt)
            qiota = cpool.tile([128, 256], f32, tag="qiota")
            nc.vector.tensor_tensor(qiota[:], qiotaf[:], qcorr[:],
                                    Op.subtract)

            call_no = 0
            done = 0
            while done < ntiles:
                nt = min(SC, ntiles - done)
                tok0 = done * 128
                ntok = nt * 128

                # ---- load superchunk ----
                xt = pool.tile([128, SC * C], f32, tag="xt")
                nc.sync.dma_start(
                    xt[:, :nt * C],
                    x_ap[tok0:tok0 + ntok, :].rearrange(
                        "(p t) c -> p (t c)", p=128))
                gt = pool.tile([128, SC * 3], f32, tag="gt")
                nc.sync.dma_start(
                    gt[:, :nt * 3],
                    geom_ap[tok0:tok0 + ntok, :].rearrange(
                        "(p t) c -> p (t c)", p=128))

                # ---- cell math (floor via int-convert + correction) ----
                def floordiv(coord_ap, tag):
                    w = pool.tile([128, SC], f32, tag=tag + "w")
                    nc.vector.tensor_scalar(w[:, :nt], coord_ap, 54.0, RECIP,
                                            Op.add, Op.mult)
                    giq = pool.tile([128, SC], i32, tag=tag + "i")
                    nc.vector.tensor_copy(giq[:, :nt], w[:, :nt])
                    gf = pool.tile([128, SC], f32, tag=tag + "f")
                    nc.vector.tensor_copy(gf[:, :nt], giq[:, :nt])
                    d = pool.tile([128, SC], f32, tag=tag + "d")
                    nc.vector.tensor_tensor(d[:, :nt], gf[:, :nt], w[:, :nt],
                                            Op.is_gt)
                    g = pool.tile([128, SC], f32, tag=tag + "g")
                    nc.vector.tensor_tensor(g[:, :nt], gf[:, :nt],
                                            d[:, :nt], Op.subtract)
                    return g

                gx = floordiv(gt[:, 0:nt * 3:3], "gx")
                gy = floordiv(gt[:, 1:nt * 3:3], "gy")
                cell = pool.tile([128, SC], f32, tag="cell")
                nc.vector.tensor_scalar(cell[:, :nt], gx[:, :nt], 360.0,
                                        None, Op.mult)
                nc.vector.tensor_tensor(cell[:, :nt], cell[:, :nt],
                                        gy[:, :nt], Op.add)
                nc.vector.tensor_scalar(cell[:, :nt], cell[:, :nt], 0.0,
                                        float(NCELL - 1), Op.max, Op.min)
                # quad = cell>>2, r4 = cell & 3
                quad = pool.tile([128, SC], f32, tag="quad")
                qi = pool.tile([128, SC], i32, tag="qi")
                qtrue = pool.tile([128, SC], f32, tag="qtrue")
                nc.vector.tensor_scalar(qtrue[:, :nt], cell[:, :nt], 0.25,
                                        None, Op.mult)
                nc.vector.tensor_copy(qi[:, :nt], qtrue[:, :nt])
                nc.vector.tensor_copy(quad[:, :nt], qi[:, :nt])
                qd = pool.tile([128, SC], f32, tag="qd")
                nc.vector.tensor_tensor(qd[:, :nt], quad[:, :nt],
                                        qtrue[:, :nt], Op.is_gt)
                nc.vector.tensor_tensor(quad[:, :nt], quad[:, :nt],
                                        qd[:, :nt], Op.subtract)
                r4 = pool.tile([128, SC], f32, tag="r4")
                nc.vector.tensor_scalar(r4[:, :nt], quad[:, :nt], -4.0,
                                        None, Op.mult)
                nc.vector.tensor_tensor(r4[:, :nt], r4[:, :nt],
                                        cell[:, :nt], Op.add)

                # ---- bf16 features ----
                xbf = pool.tile([128, SC * C], bf16, tag="xbf")
                nc.vector.tensor_copy(xbf[:, :nt * C], xt[:, :nt * C])

                # ---- per-token quarter-placed payloads, all tiles ----
                ppay = spool.tile([128, SC * E], bf16, tag="ppay")
                mskA = pool.tile([128, SC * E], bf16, tag="mskA")
                nc.vector.tensor_tensor(
                    mskA[:, :nt * E].rearrange("p (t e) -> p t e", e=E),
                    qiota[:].rearrange("p e -> p () e").broadcast_to(
                        [128, nt, E]),
                    r4[:, :nt].rearrange("p t -> p t ()").broadcast_to(
                        [128, nt, E]),
                    Op.is_equal)
                nc.vector.tensor_tensor(
                    ppay[:, :nt * E].rearrange("p (t q c) -> p t q c",
                                               q=4, c=C),
                    mskA[:, :nt * E].rearrange("p (t q c) -> p t q c",
                                               q=4, c=C),
                    xbf[:, :nt * C].rearrange("p (t c) -> p t () c",
                                              c=C).broadcast_to(
                                                  [128, nt, 4, C]),
                    Op.mult)

                # ---- QROWS: quad rows replicated to all partitions ----
                psT = ppoolT.tile([128, 128], f32, tag="psT")
                nc.tensor.matmul(psT[0:nt, :], quad[:, :nt], ident[:])
                crow = pool.tile([128, 128], f32, tag="crow")
                nc.vector.tensor_copy(crow[0:nt, :], psT[0:nt, :])
                qrows = qpool.tile([128, SC * 128], f32, tag="qrows")
                nw = nt * 128
                nc.sync.dma_start(qrows[0:1, :nw], crow[0:nt, :])
                nc.sync.dma_start(qrows[1:2, :nw], qrows[0:1, :nw])
                nc.sync.dma_start(qrows[2:4, :nw], qrows[0:2, :nw])
                nc.sync.dma_start(qrows[4:8, :nw], qrows[0:4, :nw])
                nc.sync.dma_start(qrows[8:16, :nw], qrows[0:8, :nw])
                nc.sync.dma_start(qrows[16:32, :nw], qrows[0:16, :nw])
                nc.sync.dma_start(qrows[32:64, :nw], qrows[0:32, :nw])
                nc.sync.dma_start(qrows[64:128, :nw], qrows[0:64, :nw])

                # ---- batched E matrices, ranks (3 wide DVE ops) ----
                # Eall[p, t, j] = (quad_t[j] == quad_t[p]) for all tiles
                Eall = epool.tile([128, SC * 128], bf16, tag="Eall")
                nc.vector.tensor_tensor(
                    Eall[:, :nt * 128].rearrange("p (t j) -> p t j", j=128),
                    qrows[:, :nt * 128].rearrange("p (t j) -> p t j",
                                                  j=128),
                    quad[:, :nt].rearrange("p t -> p t ()").broadcast_to(
                        [128, nt, 128]),
                    Op.is_equal)
                escr = epool.tile([128, SC * 128], bf16, tag="escr")
                nc.vector.tensor_tensor(
                    escr[:, :nt * 128].rearrange("p (t j) -> p t j", j=128),
                    Eall[:, :nt * 128].rearrange("p (t j) -> p t j", j=128),
                    ltri[:].rearrange("p j -> p () j").broadcast_to(
                        [128, nt, 128]),
                    Op.mult)
                rankc = pool.tile([128, SC], f32, tag="rankc")
                nc.vector.tensor_reduce(
                    rankc[:, :nt].rearrange("p t -> p t ()"),
                    escr[:, :nt * 128].rearrange("p (t j) -> p t j", j=128),
                    AX.X, Op.add)

                # ---- per tile: quad-row merge (one matmul each) ----
                pay = spool.tile([128, SC * E], bf16, tag="pay")
                for t0 in range(0, nt, 4):
                    g = min(4, nt - t0)
                    psM4 = ppoolM.tile([128, 4 * E], f32, tag="psM4")
                    for t in range(t0, t0 + g):
                        nc.tensor.matmul(
                            psM4[:, (t - t0) * E:(t - t0 + 1) * E],
                            Eall[:, t * 128:(t + 1) * 128],
                            ppay[:, t * E:(t + 1) * E])
                    nc.vector.tensor_copy(
                        pay[:, t0 * E:(t0 + g) * E], psM4[:, :g * E])

                # ---- idx: quad if rank==0 else DUMP ----
                isz = pool.tile([128, SC], f32, tag="isz")
                nc.vector.tensor_scalar(isz[:, :nt], rankc[:, :nt], 0.0,
                                        None, Op.is_equal)
                idxf = pool.tile([128, SC], f32, tag="idxf")
                nc.vector.tensor_scalar(idxf[:, :nt], quad[:, :nt],
                                        float(DUMP), None, Op.subtract)
                nc.vector.tensor_tensor(idxf[:, :nt], idxf[:, :nt],
                                        isz[:, :nt], Op.mult)
                nc.vector.tensor_scalar(idxf[:, :nt], idxf[:, :nt],
                                        float(DUMP), None, Op.add)
                idx16 = pool.tile([128, SC], i16, tag="idx16")
                nc.vector.tensor_copy(idx16[:, :nt], idxf[:, :nt])

                # ---- wrapped idx for the whole superchunk ----
                ncol = 8 * nt
                idxw = spool.tile([128, 8 * SC], i16, tag="idxw")
                for j in range(8):
                    nc.sync.dma_start(
                        idxw[0:16, j:ncol:8],
                        idx16[16 * j:16 * j + 16, :nt])
                nc.sync.dma_start(idxw[16:32, :ncol], idxw[0:16, :ncol])
                nc.sync.dma_start(idxw[32:64, :ncol], idxw[0:32, :ncol])
                nc.sync.dma_start(idxw[64:128, :ncol], idxw[0:64, :ncol])

                # ---- scatter calls: one per tile, rotating tables ----
                if not os.environ.get("V4_SKIP_SCATTER"):
                    for t in range(nt):
                        nc.gpsimd.dma_scatter_add(
                            tabs[call_no % NTAB].ap().rearrange(
                                "(r e) -> r e", e=E),
                            pay[:, t * E:(t + 1) * E].rearrange(
                                "p (t e) -> p t e", e=E),
                            idxw[:, t * 8:(t + 1) * 8],
                            128, 128, E, queue_num=call_no % 4)
                        call_no += 1

                done += nt

            # ---- reduce the 4 tables -> output (bf16) ----
            FW = 2048
            off = 0
            while off < zw:
                w = min(FW, zw - off)
                acc = fpool.tile([128, FW], f32, tag="facc")
                s0 = fpool.tile([128, FW], bf16, tag="fs0")
                nc.sync.dma_start(
                    s0[:, :w],
                    tabs[0].ap().rearrange("(p t) -> p t", p=128)[
                        :, off:off + w])
                nc.vector.tensor_copy(acc[:, :w], s0[:, :w])
                for r in range(1, NTAB):
                    sr = fpool.tile([128, FW], bf16, tag=f"fs{r}")
                    nc.sync.dma_start(
                        sr[:, :w],
                        tabs[r].ap().rearrange("(p t) -> p t", p=128)[
                            :, off:off + w])
                    nc.vector.tensor_tensor(acc[:, :w], acc[:, :w],
                                            sr[:, :w], Op.add)
                ob = fpool.tile([128, FW], bf16, tag="fob")
                nc.vector.tensor_copy(ob[:, :w], acc[:, :w])
                nc.sync.dma_start(
                    out_d.ap().rearrange("(p t) -> p t", p=128)[
                        :, off:off + w], ob[:, :w])
                off += w

    nc.compile()
    return nc


def kernel(geom_feats: np.ndarray, x: np.ndarray) -> np.ndarray:
    geom_feats = np.ascontiguousarray(geom_feats, dtype=np.float32)
    x = np.ascontiguousarray(x, dtype=np.float32)
    g2 = geom_feats.reshape(NP_TOTAL, 3)
    x2 = x.reshape(NP_TOTAL, C)

    if "nc" not in _cache:
        _cache["nc"] = build_program()
    nc = _cache["nc"]

    in_maps = []
    for c in range(NCORES):
        sl = slice(c * NP_CORE, (c + 1) * NP_CORE)
        in_maps.append({"geom": g2[sl], "x": x2[sl]})

    res = run_bass_kernel_spmd(nc, in_maps, core_ids=list(range(NCORES)))

    total = np.zeros((NROW * E,), np.float32)
    for c in range(NCORES):
        total += np.asarray(res.results[c]["tab"]).astype(np.float32)
    grid = total.reshape(NROW, E)[:NQUAD].reshape(NCELL, C)
    out = grid.reshape(H, W, C).transpose(2, 0, 1)[None]
    return np.ascontiguousarray(out)
